# revision 1
# baseline (speedup 1.0000x reference)
"""Trainium2 Bass kernel for nn_MC3DAD_ONNX_48146583388946 (retrieval_knn).

Per batch (one NeuronCore per batch, B=8):
  - pcd [4096, 3] -> pairwise -d^2 via a K=5 augmented matmul on TensorE
  - top-8 per row via the VectorE max8 instruction -> v5 = 5th-largest -d^2
  - the same matmul quarters are recomputed (deterministic hardware ->
    bit-identical values) and compared against v5 with a per-partition
    is_ge, so the inclusive mask selects exactly the 5 nearest neighbors
  - the [i, j] mask is transposed per 128-col block on TensorE, and a
    K=128 mask-matmul accumulates masked sums of [x, y, z, |p|^2, 1]
  - curvature = trace / sum(trace), trace from the covariance identity
    trace = (S_sq - |S_xyz|^2 / c) / (c - 1)  with c the selected count
    (c-normalization keeps exact-fp-tie rows close to the reference).

Coordinates are centered per batch on the host (translation-invariant
covariance) to avoid fp32 cancellation in the trace identity.
"""

import numpy as np
from contextlib import ExitStack

import concourse.bass as bass
import concourse.bacc as bacc
import concourse.mybir as mybir
import concourse.tile as tile
from concourse.bass_utils import run_bass_kernel_spmd

f32 = mybir.dt.float32
AF = mybir.ActivationFunctionType
ALU = mybir.AluOpType

N = 4096
B = 8
QW = 1024                      # matmul quarter width


def build_device_kernel(tc, ga_d, gb_d, pf_d, id_d, curv_d, cnt_d, n=N):
    nc = tc.nc
    ns = n // 128
    nq = n // QW                # quarters per slab row
    IH = n // 2                 # finalize half width
    with ExitStack() as ctx:
        cpool = ctx.enter_context(tc.tile_pool(name="consts", bufs=1))

        gat = cpool.tile([128, n], f32, tag="gat")
        gbt = cpool.tile([128, n], f32, tag="gbt")
        pf = cpool.tile([128, ns * 5], f32, tag="pf")
        ident = cpool.tile([128, 128], f32, tag="ident")
        s_all = cpool.tile([5, n], f32, tag="s_all")

        nc.sync.dma_start(pf[:, :], pf_d[:, :])
        nc.sync.dma_start(ident[:, :], id_d[:, :])
        for r in range(4):
            nc.sync.dma_start(gat[32 * r:32 * r + 5, :], ga_d[0:5, :])
            nc.sync.dma_start(gbt[32 * r:32 * r + 5, :], gb_d[0:5, :])

        def cdist_quarter(dst, s, q):
            """dst [128, QW] psum <- -d^2 for rows i in slab s, cols j in
            quarter q. One matmul per 512-wide psum bank; distinct row
            groups let consecutive matmuls run concurrently in the array."""
            for h in range(QW // 512):
                r = (q * (QW // 512) + h) % 4
                j0 = q * QW + h * 512
                nc.tensor.matmul(
                    dst[:, h * 512:(h + 1) * 512],
                    gbt[32 * r:32 * r + 5, s * 128:(s + 1) * 128],
                    gat[32 * r:32 * r + 5, j0:j0 + 512],
                    start=True, stop=True,
                    tile_position=(32 * r, 0),
                )

        with tc.tile_pool(name="dpsum", bufs=2, space="PSUM") as dp, \
             tc.tile_pool(name="tpsum", bufs=2, space="PSUM") as tp, \
             tc.tile_pool(name="spsum", bufs=2, space="PSUM") as sp, \
             tc.tile_pool(name="work", bufs=3) as wp, \
             tc.tile_pool(name="mwork", bufs=2) as mwp, \
             tc.tile_pool(name="twork", bufs=6) as twp:
            for s in range(ns):
                # ---- pass A: top-8 scan of row slab s ----
                m8all = wp.tile([128, 8 * nq], f32, tag="m8all")
                for q in range(nq):
                    d1 = dp.tile([128, QW], f32, tag="d1", name=f"d1_{s}_{q}")
                    cdist_quarter(d1, s, q)
                    nc.vector.max(m8all[:, q * 8:(q + 1) * 8], d1[:, :])
                m8f = wp.tile([128, 8], f32, tag="m8f")
                nc.vector.max(m8f[:, :], m8all[:, :])
                # ---- pass B: recompute, inclusive mask vs v5 ----
                mask_sb = mwp.tile([128, n], f32, tag="mask_sb")
                for q in range(nq):
                    d2 = dp.tile([128, QW], f32, tag="d1", name=f"d2_{s}_{q}")
                    cdist_quarter(d2, s, q)
                    nc.vector.tensor_scalar(
                        mask_sb[:, q * QW:(q + 1) * QW], d2[:, :],
                        m8f[:, 4:5], None, op0=ALU.is_ge)
                # ---- transpose mask blocks + masked-sum matmul ----
                s_slab = sp.tile([5, 128], f32, tag="s_slab",
                                 name=f"s_slab{s}")
                for g in range(ns // 4):        # groups of 4 transposes
                    txp = tp.tile([128, 512], f32, tag="txp",
                                  name=f"txp{s}_{g}")
                    for u in range(4):
                        t = g * 4 + u
                        nc.tensor.transpose(
                            txp[:, u * 128:(u + 1) * 128],
                            mask_sb[:, t * 128:(t + 1) * 128],
                            ident[:, :])
                    mskT = twp.tile([128, 512], f32, tag="mskT",
                                    name=f"mskT{s}_{g}")
                    nc.scalar.activation(mskT[:, :], txp[:, :], AF.Copy)
                    for u in range(4):
                        t = g * 4 + u
                        nc.tensor.matmul(
                            s_slab[:, :],
                            pf[:, t * 5:(t + 1) * 5],
                            mskT[:, u * 128:(u + 1) * 128],
                            start=(t == 0), stop=(t == ns - 1),
                        )
                nc.scalar.activation(s_all[:, s * 128:(s + 1) * 128],
                                     s_slab[:, :], AF.Copy)

        # ---------------- finalize ----------------
        with tc.tile_pool(name="fin", bufs=1) as finp:
            tr_rows = []
            den_parts = []
            for ih in range(2):
                sl = slice(ih * IH, (ih + 1) * IH)
                sqr = finp.tile([3, IH], f32, tag="sqr")
                nc.scalar.activation(sqr[:, :], s_all[0:3, sl], AF.Square)
                q = finp.tile([1, IH], f32, tag="q")
                nc.vector.tensor_copy(q[0:1, :], sqr[0:1, :])
                nc.gpsimd.dma_start(q[0:1, :], sqr[1:2, :], accum_op=ALU.add)
                nc.gpsimd.dma_start(q[0:1, :], sqr[2:3, :], accum_op=ALU.add)
                ssq0 = finp.tile([1, IH], f32, tag="ssq0")
                cnt0 = finp.tile([1, IH], f32, tag="cnt0")
                nc.sync.dma_start(ssq0[0:1, :], s_all[3:4, sl])
                nc.sync.dma_start(cnt0[0:1, :], s_all[4:5, sl])
                nc.sync.dma_start(cnt_d[0:1, sl], cnt0[0:1, :])
                rc = finp.tile([1, IH], f32, tag="rc")
                rc1 = finp.tile([1, IH], f32, tag="rc1")
                nc.vector.reciprocal(rc[0:1, :], cnt0[0:1, :])
                nc.scalar.activation(cnt0[0:1, :], cnt0[0:1, :], AF.Copy,
                                     bias=-1.0)
                nc.vector.reciprocal(rc1[0:1, :], cnt0[0:1, :])
                nc.vector.tensor_mul(q[0:1, :], q[0:1, :], rc[0:1, :])
                nc.vector.tensor_sub(ssq0[0:1, :], ssq0[0:1, :], q[0:1, :])
                tr_row = cpool.tile([1, IH], f32, tag=f"tr_row{ih}",
                                    name=f"tr_row{ih}")
                nc.vector.tensor_mul(tr_row[0:1, :], ssq0[0:1, :], rc1[0:1, :])
                den = finp.tile([1, 1], f32, tag=f"den{ih}", name=f"den{ih}")
                nc.vector.reduce_sum(den[0:1, :], tr_row[0:1, :],
                                     axis=mybir.AxisListType.X)
                tr_rows.append(tr_row)
                den_parts.append(den)

            dsum = finp.tile([1, 1], f32, tag="dsum")
            nc.vector.tensor_add(dsum[0:1, :], den_parts[0][0:1, :],
                                 den_parts[1][0:1, :])
            nc.vector.tensor_scalar_add(dsum[0:1, :], dsum[0:1, :], 1e-8)
            rden = finp.tile([1, 1], f32, tag="rden")
            nc.vector.reciprocal(rden[0:1, :], dsum[0:1, :])
            for ih in range(2):
                nc.vector.tensor_scalar_mul(tr_rows[ih][0:1, :],
                                            tr_rows[ih][0:1, :],
                                            rden[0:1, :])
                nc.sync.dma_start(curv_d[0:1, ih * IH:(ih + 1) * IH],
                                  tr_rows[ih][0:1, :])


def build_nc(n=N):
    nc = bacc.Bacc("TRN2", target_bir_lowering=False, debug=False,
                   enable_asserts=False, num_devices=B)
    ns = n // 128
    ga_d = nc.dram_tensor("ga", [6, n], f32, kind="ExternalInput").ap()
    gb_d = nc.dram_tensor("gb", [6, n], f32, kind="ExternalInput").ap()
    pf_d = nc.dram_tensor("pf", [128, ns * 5], f32, kind="ExternalInput").ap()
    id_d = nc.dram_tensor("ident", [128, 128], f32, kind="ExternalInput").ap()
    curv_d = nc.dram_tensor("curv", [1, n], f32, kind="ExternalOutput").ap()
    cnt_d = nc.dram_tensor("cnt", [1, n], f32, kind="ExternalOutput").ap()
    with tile.TileContext(nc) as tc:
        build_device_kernel(tc, ga_d, gb_d, pf_d, id_d, curv_d, cnt_d, n=n)
    nc.compile()
    return nc


def host_inputs(p, n=N):
    """Per-batch host prep. p: [n, 3] float32 (uncentered)."""
    ns = n // 128
    mu = p.mean(axis=0, dtype=np.float32)
    p = (p - mu).astype(np.float32)
    x, y, z = p[:, 0].copy(), p[:, 1].copy(), p[:, 2].copy()
    sq = (x * x + y * y) + z * z
    one = np.ones(n, np.float32)
    ga = np.ascontiguousarray(np.stack([x, y, z, sq, one, one]))
    gb = np.ascontiguousarray(
        np.stack([2 * x, 2 * y, 2 * z, -one, -sq, np.zeros(n, np.float32)]))
    pfm = np.stack([x, y, z, sq, one], axis=1)            # [n, 5]
    pfm = np.ascontiguousarray(
        pfm.reshape(ns, 128, 5).transpose(1, 0, 2).reshape(128, ns * 5))
    ident = np.eye(128, dtype=np.float32)
    return {"ga": ga, "gb": gb, "pf": pfm, "ident": ident}


_NC_CACHE = {}


def kernel(pcd, k):
    assert int(k) == 5, f"kernel hardcodes k=5, got {k}"
    pcd = np.asarray(pcd, dtype=np.float32)
    assert pcd.shape == (B, N, 3), pcd.shape
    if N not in _NC_CACHE:
        _NC_CACHE[N] = build_nc(N)
    nc = _NC_CACHE[N]
    in_maps = [host_inputs(pcd[b]) for b in range(B)]
    res = run_bass_kernel_spmd(nc, in_maps, core_ids=list(range(B)))
    out = np.stack([r["curv"].reshape(N) for r in res.results])
    return out.astype(np.float32)


if __name__ == "__main__":
    rng = np.random.default_rng(0)
    pcd = rng.standard_normal((B, N, 3)).astype(np.float32)
    out = kernel(pcd, 5)
    print("kernel output", out.shape, out.dtype, out[0, :4])

